# revision 1
# baseline (speedup 1.0000x reference)
"""GCN feature extractor (GCNConv + BatchNorm1d + ReLU) as a Trainium2 Bass kernel.

Distribution (8 NeuronCores):
  - Nodes are sharded row-wise across the 8 cores (graph/data parallel).
  - Each core computes m = deg^-1/2 * (x @ W) for its own node shard (PE matmul),
    casts to bf16, and the shards are AllGather'd into a replicated full
    message table in each core's DRAM.
  - Each core owns the edges whose TARGET falls in its shard.  Per 128-target
    tile it bulk-gathers the source messages with the GpSimd descriptor-
    generated gather DMA (dma_gather), builds one-hot target-selection
    matrices on the vector engine, and reduces on the tensor engine:
        agg[f, t] += G[edge, f]^T @ onehot[edge, t]   (PSUM fp32 accumulate)
  - Self loops are folded in as ordinary edges; the symmetric normalization
    factorizes as deg^-1/2[src] (folded into m) and deg^-1/2[tgt] (applied
    per tile).  The +bias term cancels exactly under BatchNorm and is dropped.
  - BatchNorm statistics are computed per-feature on the fly (features live on
    partitions), AllReduce'd across cores, and applied fused with ReLU on the
    scalar engine.  Output is written feature-major; the host transposes.
"""

import sys

sys.path.insert(0, "/opt/trn_rl_repo")

import numpy as np
import ml_dtypes

import os
import concourse.bass as bass
import concourse.tile as tile
from concourse import bacc, mybir, library_config
from concourse.bass_utils import run_bass_kernel_spmd

N_CORES = 8
P = 128
GK = 8  # gather-DMA granularity: blocks (of 128 edges) per dma_gather call (1024 idx HW limit)
BN_EPS = 1e-5
dt = mybir.dt


# ---------------------------------------------------------------- host prep
def _plan_and_pack(x, edge_index, W, gamma, beta):
    N, IN = x.shape
    HID = W.shape[1]
    assert HID == P and IN % P == 0
    shard = (N + N_CORES - 1) // N_CORES          # nodes per core (last may be short)
    PS = ((shard + P - 1) // P) * P               # padded shard rows
    NT = PS // P                                  # 128-target tiles per core
    half = (N_CORES // 2) * PS                    # window split of the gather table
    assert half < 2 ** 15, "int16 gather index overflow"

    row = np.asarray(edge_index[0], dtype=np.int64)
    col = np.asarray(edge_index[1], dtype=np.int64)
    E = row.shape[0]

    deg = np.bincount(col, minlength=N).astype(np.float64) + 1.0
    dis = (1.0 / np.sqrt(deg)).astype(np.float32)

    # padded-table coordinates of each node
    tbl = (np.arange(N) // shard) * PS + (np.arange(N) % shard)

    # append self loops, sort edges by target
    allr = np.concatenate([row, np.arange(N)])
    allc = np.concatenate([col, np.arange(N)])
    order = np.argsort(allc, kind="stable")
    allr = allr[order]
    allc = allc[order]
    src_tbl_all = tbl[allr].astype(np.int32)

    # per-core / per-tile / per-window edge lists
    # ec[c][t][w] = np.array of window-relative source table rows
    tile_of = allc // shard * NT + (allc % shard) // P
    tloc_of = (allc % shard) % P
    win_of = (src_tbl_all >= half).astype(np.int64)

    n_tiles_total = N_CORES * NT
    key = tile_of * 2 + win_of
    sort2 = np.argsort(key, kind="stable")
    src_sorted = src_tbl_all[sort2]
    tloc_sorted = tloc_of[sort2].astype(np.int32)
    key_sorted = key[sort2]
    bounds = np.searchsorted(key_sorted, np.arange(n_tiles_total * 2 + 1))

    # shared (max over cores) block counts per (tile, window)
    nb = np.zeros((N_CORES, NT, 2), np.int64)
    for c in range(N_CORES):
        for t in range(NT):
            for w in range(2):
                k = (c * NT + t) * 2 + w
                cnt = bounds[k + 1] - bounds[k]
                nb[c, t, w] = (cnt + P - 1) // P
    nbmax = nb.max(axis=0)                         # [NT, 2]
    # stream block offsets: stream w blocks of tile t start at soff[t, w]
    soff = np.zeros((NT, 2), np.int64)
    tb = [0, 0]
    for t in range(NT):
        for w in range(2):
            soff[t, w] = tb[w]
            tb[w] += nbmax[t, w]
    TB0, TB1 = int(tb[0]), int(tb[1])
    ncalls = [(TB0 + GK - 1) // GK, (TB1 + GK - 1) // GK]
    TBpad = [ncalls[0] * GK, ncalls[1] * GK]

    per_core = []
    for c in range(N_CORES):
        streams = [np.zeros(TBpad[w] * P, np.int32) for w in range(2)]
        tgtl = [-np.ones((P, TBpad[w]), np.float32) for w in range(2)]
        for t in range(NT):
            for w in range(2):
                k = (c * NT + t) * 2 + w
                lo, hi = bounds[k], bounds[k + 1]
                srcs = src_sorted[lo:hi] - w * half
                tl = tloc_sorted[lo:hi]
                b0 = soff[t, w]
                streams[w][b0 * P: b0 * P + (hi - lo)] = srcs
                tg = tgtl[w]
                for j in range(hi - lo):
                    tg[j % P, b0 + j // P] = tl[j]
        # pack gather indices: per call [128, GK*128/16] int16, idx j -> [16c + j%16, j//16]
        idxs = []
        for w in range(2):
            s16 = streams[w].astype(np.int16)
            a = s16.reshape(ncalls[w], GK * P // 16, 16).transpose(0, 2, 1)  # [calls, 16, cols]
            a = np.tile(a, (1, 8, 1))                                        # replicate to 128 partitions
            idxs.append(np.ascontiguousarray(a.transpose(1, 0, 2).reshape(P, -1)))

        lo_n = c * shard
        hi_n = min((c + 1) * shard, N)
        ns = hi_n - lo_n
        xs = np.zeros((IN, PS), np.float32)
        xs[:, :ns] = x[lo_n:hi_n].T
        dis_s = np.zeros(PS, np.float32)
        dis_s[:ns] = dis[lo_n:hi_n]
        per_core.append({
            "xT": xs,
            "disb": np.ascontiguousarray(np.tile(dis_s[None, :], (P, 1))),
            "disk": np.ascontiguousarray(dis_s.reshape(NT, P).T),   # [128, NT]
            "idx0": idxs[0], "idx1": idxs[1],
            "tgtl0": tgtl[0].astype(ml_dtypes.bfloat16),
            "tgtl1": tgtl[1].astype(ml_dtypes.bfloat16),
            "W": np.ascontiguousarray(W.astype(np.float32)),
            "iota": np.ascontiguousarray(
                np.tile(np.arange(P, dtype=np.float32), (P, 1)).astype(ml_dtypes.bfloat16)),
            "gamma": np.ascontiguousarray(gamma.astype(np.float32).reshape(P, 1)),
            "beta": np.ascontiguousarray(beta.astype(np.float32).reshape(P, 1)),
        })

    plan = {
        "N": N, "IN": IN, "PS": PS, "NT": NT, "half": half, "shard": shard,
        "nbmax": nbmax, "soff": soff, "TB": [TB0, TB1], "ncalls": ncalls,
        "KC": IN // P,
    }
    return plan, per_core


# ---------------------------------------------------------------- bass build
def _build(plan):
    N, IN, PS, NT = plan["N"], plan["IN"], plan["PS"], plan["NT"]
    KC = plan["KC"]
    half = plan["half"]
    nbmax, soff = plan["nbmax"], plan["soff"]
    ncalls = plan["ncalls"]
    NIDX = GK * P

    nc = bacc.Bacc("TRN2", target_bir_lowering=False, debug=False,
                   num_devices=N_CORES)
    t_xT = nc.dram_tensor("xT", [IN, PS], dt.float32, kind="ExternalInput").ap()
    t_W = nc.dram_tensor("W", [IN, P], dt.float32, kind="ExternalInput").ap()
    t_disb = nc.dram_tensor("disb", [P, PS], dt.float32, kind="ExternalInput").ap()
    t_disk = nc.dram_tensor("disk", [P, NT], dt.float32, kind="ExternalInput").ap()
    t_idx = [nc.dram_tensor(f"idx{w}", [P, ncalls[w] * NIDX // 16], dt.int16,
                            kind="ExternalInput").ap() for w in range(2)]
    t_tgtl = [nc.dram_tensor(f"tgtl{w}", [P, ncalls[w] * GK], dt.bfloat16,
                             kind="ExternalInput").ap() for w in range(2)]
    t_iota = nc.dram_tensor("iota", [P, P], dt.bfloat16, kind="ExternalInput").ap()
    t_gamma = nc.dram_tensor("gamma", [P, 1], dt.float32, kind="ExternalInput").ap()
    t_beta = nc.dram_tensor("beta", [P, 1], dt.float32, kind="ExternalInput").ap()
    t_out = nc.dram_tensor("out_t", [P, PS], dt.float32, kind="ExternalOutput").ap()

    INV_N = 1.0 / N

    STAGE = int(os.environ.get("K_STAGE", "99"))
    with tile.TileContext(nc) as tc:
        nc.gpsimd.load_library(library_config.mlp)
        with tc.tile_pool(name="consts", bufs=1) as cst, \
             tc.tile_pool(name="xtp", bufs=3) as xtp, \
             tc.tile_pool(name="mbp", bufs=3) as mbp, \
             tc.tile_pool(name="gp0", bufs=2) as gp0, \
             tc.tile_pool(name="gp1", bufs=2) as gp1, \
             tc.tile_pool(name="ohp", bufs=3) as ohp, \
             tc.tile_pool(name="ep", bufs=3) as ep, \
             tc.tile_pool(name="hps", bufs=2, space="PSUM") as hps, \
             tc.tile_pool(name="aps", bufs=4, space="PSUM") as aps, \
             tc.tile_pool(name="stp", bufs=1) as stp, \
             tc.tile_pool(name="dram", bufs=1, space="DRAM") as dram:

            # ---- constants to SBUF
            W_sb = cst.tile([P, KC, P], dt.float32)
            for k in range(KC):
                nc.sync.dma_start(out=W_sb[:, k, :], in_=t_W[k * P:(k + 1) * P, :])
            disk_sb = cst.tile([P, NT], dt.float32)
            nc.sync.dma_start(out=disk_sb[:], in_=t_disk[:])
            disb_sb = cst.tile([P, PS], dt.float32)
            nc.sync.dma_start(out=disb_sb[:], in_=t_disb[:])
            iota_sb = cst.tile([P, P], dt.bfloat16)
            nc.sync.dma_start(out=iota_sb[:], in_=t_iota[:])
            gamma_sb = cst.tile([P, 1], dt.float32)
            nc.sync.dma_start(out=gamma_sb[:], in_=t_gamma[:])
            beta_sb = cst.tile([P, 1], dt.float32)
            nc.sync.dma_start(out=beta_sb[:], in_=t_beta[:])
            idx_sb = [cst.tile([P, ncalls[w] * NIDX // 16], dt.int16, name=f"idx{w}")
                      for w in range(2)]
            tgtl_sb = [cst.tile([P, ncalls[w] * GK], dt.bfloat16, name=f"tg{w}")
                       for w in range(2)]
            for w in range(2):
                nc.sync.dma_start(out=idx_sb[w][:], in_=t_idx[w][:])
                nc.sync.dma_start(out=tgtl_sb[w][:], in_=t_tgtl[w][:])

            # ---- phase B: m = dis * (x @ W), bf16, own shard -> DRAM
            m_cc = dram.tile([PS, P], dt.bfloat16)
            for k in range(NT):
                xt = xtp.tile([P, KC, P], dt.float32, name="xt")
                for q in range(KC):
                    nc.sync.dma_start(
                        out=xt[:, q, :],
                        in_=t_xT[q * P:(q + 1) * P, k * P:(k + 1) * P])
                h_ps = hps.tile([P, P], dt.float32, name="hps")
                for q in range(KC):
                    nc.tensor.matmul(out=h_ps[:], lhsT=xt[:, q, :],
                                     rhs=W_sb[:, q, :],
                                     start=(q == 0), stop=(q == KC - 1))
                m_bf = mbp.tile([P, P], dt.bfloat16, name="mbf")
                nc.vector.tensor_scalar(out=m_bf[:], in0=h_ps[:],
                                        scalar1=disk_sb[:, k:k + 1], scalar2=None,
                                        op0=mybir.AluOpType.mult)
                nc.sync.dma_start(out=m_cc[k * P:(k + 1) * P, :], in_=m_bf[:])

            # ---- replicate m across cores
            m_full = dram.tile([N_CORES * PS, P], dt.bfloat16, addr_space="Shared")
            if STAGE >= 2:
                nc.gpsimd.collective_compute(
                    "AllGather", mybir.AluOpType.bypass,
                    replica_groups=[list(range(N_CORES))],
                    ins=[m_cc[:]], outs=[m_full[:]])
            else:
                for _c in range(N_CORES):
                    nc.sync.dma_start(out=m_full[_c * PS:(_c + 1) * PS, :], in_=m_cc[:])

            # ---- gather pipelines (two int16 windows)
            g_tiles = [[], []]
            gpools = [gp0, gp1]
            for w in range(2) if STAGE >= 3 else []:
                base = w * half
                for cidx in range(ncalls[w]):
                    gt = gpools[w].tile([P, GK, P], dt.bfloat16, name=f"g{w}")
                    nc.gpsimd.dma_gather(
                        out_ap=gt[:],
                        in_ap=m_full[base:base + half, :],
                        idxs_ap=idx_sb[w][:, cidx * NIDX // 16:(cidx + 1) * NIDX // 16],
                        num_idxs=NIDX, num_idxs_reg=NIDX, elem_size=P)
                    g_tiles[w].append(gt)

            # ---- aggregation + stats
            s1_parts = stp.tile([P, NT], dt.float32)
            s2_parts = stp.tile([P, NT], dt.float32)
            opre_all = stp.tile([P, NT, P], dt.float32)
            OHMAX = int(nbmax.sum(axis=1).max())
            for t in range(NT):
                if STAGE < 4:
                    op_t = opre_all[:, t, :]
                    nc.vector.memset(op_t, 0.125)
                    nc.vector.tensor_reduce(out=s1_parts[:, t:t + 1], in_=op_t,
                                            axis=mybir.AxisListType.X,
                                            op=mybir.AluOpType.add)
                    nc.vector.tensor_reduce(out=s2_parts[:, t:t + 1], in_=op_t,
                                            axis=mybir.AxisListType.X,
                                            op=mybir.AluOpType.add)
                    continue
                ps_t = aps.tile([P, P], dt.float32, name="agg")
                total_nb = int(nbmax[t, 0] + nbmax[t, 1])
                oh = ohp.tile([P, OHMAX, P], dt.bfloat16, name="oh")
                done = 0
                for w in range(2):
                    nbw = int(nbmax[t, w])
                    if nbw == 0:
                        continue
                    b0 = int(soff[t, w])
                    nc.vector.tensor_tensor(
                        out=oh[:, done:done + nbw, :],
                        in0=tgtl_sb[w][:, b0:b0 + nbw].unsqueeze(2)
                            .to_broadcast([P, nbw, P]),
                        in1=iota_sb[:].unsqueeze(1).to_broadcast([P, nbw, P]),
                        op=mybir.AluOpType.is_equal)
                    for b in range(nbw):
                        j = b0 + b
                        gt = g_tiles[w][j // GK]
                        nc.tensor.matmul(
                            out=ps_t[:], lhsT=gt[:, j % GK, :],
                            rhs=oh[:, done + b, :],
                            start=(done + b == 0),
                            stop=(done + b == total_nb - 1))
                    done += nbw
                op_t = opre_all[:, t, :]
                nc.vector.tensor_mul(out=op_t, in0=ps_t[:],
                                     in1=disb_sb[:, t * P:(t + 1) * P])
                nc.vector.tensor_reduce(out=s1_parts[:, t:t + 1], in_=op_t,
                                        axis=mybir.AxisListType.X,
                                        op=mybir.AluOpType.add)
                sq_t = ep.tile([P, P], dt.float32, name="sq")
                nc.scalar.activation(out=sq_t[:], in_=op_t,
                                     func=mybir.ActivationFunctionType.Square)
                nc.vector.tensor_reduce(out=s2_parts[:, t:t + 1], in_=sq_t[:],
                                        axis=mybir.AxisListType.X,
                                        op=mybir.AluOpType.add)

            # ---- BN stats allreduce + affine coefficients
            st_sb = stp.tile([P, 2], dt.float32)
            nc.vector.tensor_reduce(out=st_sb[:, 0:1], in_=s1_parts[:],
                                    axis=mybir.AxisListType.X, op=mybir.AluOpType.add)
            nc.vector.tensor_reduce(out=st_sb[:, 1:2], in_=s2_parts[:],
                                    axis=mybir.AxisListType.X, op=mybir.AluOpType.add)
            st_in = dram.tile([P, 2], dt.float32)
            st_out = dram.tile([P, 2], dt.float32, addr_space="Shared")
            st2_sb = stp.tile([P, 2], dt.float32)
            if STAGE >= 5:
                nc.sync.dma_start(out=st_in[:], in_=st_sb[:])
                nc.gpsimd.collective_compute(
                    "AllReduce", mybir.AluOpType.add,
                    replica_groups=[list(range(N_CORES))],
                    ins=[st_in[:]], outs=[st_out[:]])
                nc.sync.dma_start(out=st2_sb[:], in_=st_out[:])
            else:
                nc.vector.tensor_scalar_mul(st2_sb[:], st_sb[:], float(N_CORES))

            mean = stp.tile([P, 1], dt.float32)
            nc.vector.tensor_scalar_mul(mean[:], st2_sb[:, 0:1], INV_N)
            var = stp.tile([P, 1], dt.float32)
            nc.vector.tensor_scalar_mul(var[:], st2_sb[:, 1:2], INV_N)
            nmm = stp.tile([P, 1], dt.float32)
            nc.vector.scalar_tensor_tensor(out=nmm[:], in0=mean[:], scalar=-1.0,
                                           in1=mean[:], op0=mybir.AluOpType.mult,
                                           op1=mybir.AluOpType.mult)
            nc.vector.tensor_add(out=var[:], in0=var[:], in1=nmm[:])
            nc.vector.tensor_scalar_add(var[:], var[:], BN_EPS)
            std = stp.tile([P, 1], dt.float32)
            nc.scalar.activation(out=std[:], in_=var[:],
                                 func=mybir.ActivationFunctionType.Sqrt)
            rstd = stp.tile([P, 1], dt.float32)
            nc.vector.reciprocal(out=rstd[:], in_=std[:])
            A = stp.tile([P, 1], dt.float32)
            nc.vector.tensor_mul(out=A[:], in0=gamma_sb[:], in1=rstd[:])
            B = stp.tile([P, 1], dt.float32)
            nc.vector.tensor_mul(out=B[:], in0=A[:], in1=mean[:])
            nc.vector.scalar_tensor_tensor(out=B[:], in0=B[:], scalar=-1.0,
                                           in1=beta_sb[:], op0=mybir.AluOpType.mult,
                                           op1=mybir.AluOpType.add)

            # ---- finalize: relu(A*x + B), write feature-major output
            for t in range(NT):
                fin = ep.tile([P, P], dt.float32, name="fin")
                nc.scalar.activation(out=fin[:], in_=opre_all[:, t, :],
                                     func=mybir.ActivationFunctionType.Relu,
                                     bias=B[:], scale=A[:])
                nc.sync.dma_start(out=t_out[:, t * P:(t + 1) * P], in_=fin[:])

    nc.compile()
    return nc


# ---------------------------------------------------------------- entrypoint
def kernel(x, edge_index, W, b, gamma, beta):
    x = np.asarray(x, dtype=np.float32)
    edge_index = np.asarray(edge_index)
    W = np.asarray(W, dtype=np.float32)
    gamma = np.asarray(gamma, dtype=np.float32)
    beta = np.asarray(beta, dtype=np.float32)
    # bias cancels exactly under BatchNorm (constant per-feature shift); unused.

    plan, per_core = _plan_and_pack(x, edge_index, W, gamma, beta)
    nc = _build(plan)
    res = run_bass_kernel_spmd(nc, per_core, list(range(N_CORES)))

    N, shard = plan["N"], plan["shard"]
    out = np.empty((N, P), np.float32)
    for c in range(N_CORES):
        lo = c * shard
        hi = min((c + 1) * shard, N)
        out[lo:hi] = res.results[c]["out_t"][:, : hi - lo].T
    return out


if __name__ == "__main__":
    rng = np.random.default_rng(0)
    N, E = 2048, 8192
    x = rng.standard_normal((N, 256), dtype=np.float32)
    ei = rng.integers(0, N, (2, E)).astype(np.int64)
    W = (rng.standard_normal((256, 128), dtype=np.float32) / 16)
    g = rng.standard_normal(128).astype(np.float32) + 1.2
    be = rng.standard_normal(128).astype(np.float32)
    got = kernel(x=x, edge_index=ei, W=W, b=np.zeros(128, np.float32), gamma=g, beta=be)

    h = x @ W
    loops = np.arange(N)
    r2 = np.concatenate([ei[0], loops]); c2 = np.concatenate([ei[1], loops])
    deg = np.bincount(c2, minlength=N).astype(np.float32)
    dis = 1.0 / np.sqrt(deg)
    out = np.zeros((N, 128), np.float32)
    np.add.at(out, c2, h[r2] * (dis[r2] * dis[c2])[:, None])
    mean = out.mean(0); var = ((out - mean) ** 2).mean(0)
    ref = np.maximum(g * (out - mean) / np.sqrt(var + BN_EPS) + be, 0)
    err = np.abs(got - ref)
    print("absmax:", err.max(), "scale:", np.abs(ref).max(),
          "rel:", err.max() / np.abs(ref).max())



# revision 4
# speedup vs baseline: 1.6347x; 1.6347x over previous
"""GCN feature extractor (GCNConv + BatchNorm1d + ReLU) as a Trainium2 Bass kernel.

Distribution (8 NeuronCores):
  - Nodes are sharded row-wise across the 8 cores (graph/data parallel).
  - Each core computes m = deg^-1/2 * (x @ W) for its own node shard (PE matmul),
    casts to bf16, and the shards are AllGather'd into a replicated full
    message table in each core's DRAM.
  - Each core owns the edges whose TARGET falls in its shard.  Per 128-target
    tile it bulk-gathers the source messages with the GpSimd descriptor-
    generated gather DMA (dma_gather), builds one-hot target-selection
    matrices on the vector engine, and reduces on the tensor engine:
        agg[f, t] += G[edge, f]^T @ onehot[edge, t]   (PSUM fp32 accumulate)
  - Self loops are folded in as ordinary edges; the symmetric normalization
    factorizes as deg^-1/2[src] (folded into m) and deg^-1/2[tgt] (applied
    per tile).  The +bias term cancels exactly under BatchNorm and is dropped.
  - BatchNorm statistics are computed per-feature on the fly (features live on
    partitions), AllReduce'd across cores, and applied fused with ReLU on the
    scalar engine.  Output is written feature-major; the host transposes.
"""

import sys

sys.path.insert(0, "/opt/trn_rl_repo")

import numpy as np
import ml_dtypes

import os
import concourse.bass as bass
import concourse.tile as tile
from concourse import bacc, mybir, library_config
from concourse.bass_utils import run_bass_kernel_spmd

N_CORES = 8
P = 128
GK = 8  # gather-DMA granularity: blocks (of 128 edges) per dma_gather call (1024 idx HW limit)
BN_EPS = 1e-5
dt = mybir.dt


# ---------------------------------------------------------------- host prep
def _plan_and_pack(x, edge_index, W, gamma, beta):
    N, IN = x.shape
    HID = W.shape[1]
    assert HID == P and IN % P == 0
    shard = (N + N_CORES - 1) // N_CORES          # nodes per core (last may be short)
    PS = ((shard + P - 1) // P) * P               # padded shard rows
    NT = PS // P                                  # 128-target tiles per core
    half = (N_CORES // 2) * PS                    # window split of the gather table
    assert half < 2 ** 15, "int16 gather index overflow"

    row = np.asarray(edge_index[0], dtype=np.int64)
    col = np.asarray(edge_index[1], dtype=np.int64)
    E = row.shape[0]

    deg = np.bincount(col, minlength=N).astype(np.float64) + 1.0
    dis = (1.0 / np.sqrt(deg)).astype(np.float32)

    # padded-table coordinates of each node
    tbl = (np.arange(N) // shard) * PS + (np.arange(N) % shard)

    # append self loops, sort edges by target
    allr = np.concatenate([row, np.arange(N)])
    allc = np.concatenate([col, np.arange(N)])
    order = np.argsort(allc, kind="stable")
    allr = allr[order]
    allc = allc[order]
    src_tbl_all = tbl[allr].astype(np.int32)

    # per-core / per-tile / per-window edge lists
    # ec[c][t][w] = np.array of window-relative source table rows
    tile_of = allc // shard * NT + (allc % shard) // P
    tloc_of = (allc % shard) % P
    win_of = (src_tbl_all >= half).astype(np.int64)

    n_tiles_total = N_CORES * NT
    key = tile_of * 2 + win_of
    sort2 = np.argsort(key, kind="stable")
    src_sorted = src_tbl_all[sort2]
    tloc_sorted = tloc_of[sort2].astype(np.int32)
    key_sorted = key[sort2]
    bounds = np.searchsorted(key_sorted, np.arange(n_tiles_total * 2 + 1))

    # shared (max over cores) block counts per (tile, window)
    nb = np.zeros((N_CORES, NT, 2), np.int64)
    for c in range(N_CORES):
        for t in range(NT):
            for w in range(2):
                k = (c * NT + t) * 2 + w
                cnt = bounds[k + 1] - bounds[k]
                nb[c, t, w] = (cnt + P - 1) // P
    nbmax = nb.max(axis=0)                         # [NT, 2]
    # stream block offsets: stream w blocks of tile t start at soff[t, w]
    soff = np.zeros((NT, 2), np.int64)
    tb = [0, 0]
    for t in range(NT):
        for w in range(2):
            soff[t, w] = tb[w]
            tb[w] += nbmax[t, w]
    TB0, TB1 = int(tb[0]), int(tb[1])
    ncalls = [(TB0 + GK - 1) // GK, (TB1 + GK - 1) // GK]
    TBpad = [ncalls[0] * GK, ncalls[1] * GK]

    per_core = []
    for c in range(N_CORES):
        streams = [np.zeros(TBpad[w] * P, np.int32) for w in range(2)]
        tgtl = [-np.ones((P, TBpad[w]), np.float32) for w in range(2)]
        for t in range(NT):
            for w in range(2):
                k = (c * NT + t) * 2 + w
                lo, hi = bounds[k], bounds[k + 1]
                srcs = src_sorted[lo:hi] - w * half
                tl = tloc_sorted[lo:hi]
                b0 = soff[t, w]
                streams[w][b0 * P: b0 * P + (hi - lo)] = srcs
                tg = tgtl[w]
                for j in range(hi - lo):
                    tg[j % P, b0 + j // P] = tl[j]
        # pack gather indices: per call [128, GK*128/16] int16, idx j -> [16c + j%16, j//16]
        idxs = []
        for w in range(2):
            s16 = streams[w].astype(np.int16)
            a = s16.reshape(ncalls[w], GK * P // 16, 16).transpose(0, 2, 1)  # [calls, 16, cols]
            a = np.tile(a, (1, 8, 1))                                        # replicate to 128 partitions
            idxs.append(np.ascontiguousarray(a.transpose(1, 0, 2).reshape(P, -1)))

        lo_n = c * shard
        hi_n = min((c + 1) * shard, N)
        ns = hi_n - lo_n
        xs = np.zeros((IN, PS), np.float32)
        xs[:, :ns] = x[lo_n:hi_n].T
        dis_s = np.zeros(PS, np.float32)
        dis_s[:ns] = dis[lo_n:hi_n]
        per_core.append({
            "xT": xs,
            "disb": np.ascontiguousarray(np.tile(dis_s[None, :], (P, 1))),
            "disk": np.ascontiguousarray(dis_s.reshape(NT, P).T),   # [128, NT]
            "idx0": idxs[0], "idx1": idxs[1],
            "tgtl0": tgtl[0].astype(ml_dtypes.bfloat16),
            "tgtl1": tgtl[1].astype(ml_dtypes.bfloat16),
            "W": np.ascontiguousarray(W.astype(np.float32)),
            "iota": np.ascontiguousarray(
                np.tile(np.arange(P, dtype=np.float32), (P, 1)).astype(ml_dtypes.bfloat16)),
            "gamma": np.ascontiguousarray(gamma.astype(np.float32).reshape(P, 1)),
            "beta": np.ascontiguousarray(beta.astype(np.float32).reshape(P, 1)),
        })

    plan = {
        "N": N, "IN": IN, "PS": PS, "NT": NT, "half": half, "shard": shard,
        "nbmax": nbmax, "soff": soff, "TB": [TB0, TB1], "ncalls": ncalls,
        "KC": IN // P,
    }
    return plan, per_core


# ---------------------------------------------------------------- bass build
def _build(plan):
    N, IN, PS, NT = plan["N"], plan["IN"], plan["PS"], plan["NT"]
    KC = plan["KC"]
    half = plan["half"]
    nbmax, soff = plan["nbmax"], plan["soff"]
    ncalls = plan["ncalls"]
    NIDX = GK * P

    nc = bacc.Bacc("TRN2", target_bir_lowering=False, debug=False,
                   num_devices=N_CORES, num_swdge_queues=4)
    t_xT = nc.dram_tensor("xT", [IN, PS], dt.float32, kind="ExternalInput").ap()
    t_W = nc.dram_tensor("W", [IN, P], dt.float32, kind="ExternalInput").ap()
    t_disb = nc.dram_tensor("disb", [P, PS], dt.float32, kind="ExternalInput").ap()
    t_disk = nc.dram_tensor("disk", [P, NT], dt.float32, kind="ExternalInput").ap()
    t_idx = [nc.dram_tensor(f"idx{w}", [P, ncalls[w] * NIDX // 16], dt.int16,
                            kind="ExternalInput").ap() for w in range(2)]
    t_tgtl = [nc.dram_tensor(f"tgtl{w}", [P, ncalls[w] * GK], dt.bfloat16,
                             kind="ExternalInput").ap() for w in range(2)]
    t_iota = nc.dram_tensor("iota", [P, P], dt.bfloat16, kind="ExternalInput").ap()
    t_gamma = nc.dram_tensor("gamma", [P, 1], dt.float32, kind="ExternalInput").ap()
    t_beta = nc.dram_tensor("beta", [P, 1], dt.float32, kind="ExternalInput").ap()
    t_out = nc.dram_tensor("out_t", [P, PS], dt.float32, kind="ExternalOutput").ap()

    INV_N = 1.0 / N

    STAGE = int(os.environ.get("K_STAGE", "99"))
    with tile.TileContext(nc) as tc:
        nc.gpsimd.load_library(library_config.mlp)
        with tc.tile_pool(name="consts", bufs=1) as cst, \
             tc.tile_pool(name="xtp", bufs=3) as xtp, \
             tc.tile_pool(name="mbp", bufs=3) as mbp, \
             tc.tile_pool(name="gp0", bufs=3) as gp0, \
             tc.tile_pool(name="gp1", bufs=3) as gp1, \
             tc.tile_pool(name="ohp", bufs=3) as ohp, \
             tc.tile_pool(name="ep", bufs=3) as ep, \
             tc.tile_pool(name="hps", bufs=2, space="PSUM") as hps, \
             tc.tile_pool(name="aps", bufs=4, space="PSUM") as aps, \
             tc.tile_pool(name="stp", bufs=1) as stp, \
             tc.tile_pool(name="dram", bufs=1, space="DRAM") as dram:

            # ---- constants to SBUF
            W_sb = cst.tile([P, KC, P], dt.float32)
            for k in range(KC):
                nc.sync.dma_start(out=W_sb[:, k, :], in_=t_W[k * P:(k + 1) * P, :])
            disk_sb = cst.tile([P, NT], dt.float32)
            nc.sync.dma_start(out=disk_sb[:], in_=t_disk[:])
            disb_sb = cst.tile([P, PS], dt.float32)
            nc.sync.dma_start(out=disb_sb[:], in_=t_disb[:])
            iota_sb = cst.tile([P, P], dt.bfloat16)
            nc.sync.dma_start(out=iota_sb[:], in_=t_iota[:])
            gamma_sb = cst.tile([P, 1], dt.float32)
            nc.sync.dma_start(out=gamma_sb[:], in_=t_gamma[:])
            beta_sb = cst.tile([P, 1], dt.float32)
            nc.sync.dma_start(out=beta_sb[:], in_=t_beta[:])
            idx_sb = [cst.tile([P, ncalls[w] * NIDX // 16], dt.int16, name=f"idx{w}")
                      for w in range(2)]
            tgtl_sb = [cst.tile([P, ncalls[w] * GK], dt.bfloat16, name=f"tg{w}")
                       for w in range(2)]
            for w in range(2):
                nc.sync.dma_start(out=idx_sb[w][:], in_=t_idx[w][:])
                nc.sync.dma_start(out=tgtl_sb[w][:], in_=t_tgtl[w][:])

            # ---- phase B: m = dis * (x @ W), bf16, own shard -> DRAM
            m_cc = dram.tile([PS, P], dt.bfloat16)
            for k in range(NT):
                xt = xtp.tile([P, KC, P], dt.float32, name="xt")
                for q in range(KC):
                    nc.sync.dma_start(
                        out=xt[:, q, :],
                        in_=t_xT[q * P:(q + 1) * P, k * P:(k + 1) * P])
                h_ps = hps.tile([P, P], dt.float32, name="hps")
                for q in range(KC):
                    nc.tensor.matmul(out=h_ps[:], lhsT=xt[:, q, :],
                                     rhs=W_sb[:, q, :],
                                     start=(q == 0), stop=(q == KC - 1))
                m_bf = mbp.tile([P, P], dt.bfloat16, name="mbf")
                nc.vector.tensor_scalar(out=m_bf[:], in0=h_ps[:],
                                        scalar1=disk_sb[:, k:k + 1], scalar2=None,
                                        op0=mybir.AluOpType.mult)
                nc.sync.dma_start(out=m_cc[k * P:(k + 1) * P, :], in_=m_bf[:])

            # ---- replicate m across cores
            m_full = dram.tile([N_CORES * PS, P], dt.bfloat16, addr_space="Shared")
            if STAGE >= 2:
                nc.gpsimd.collective_compute(
                    "AllGather", mybir.AluOpType.bypass,
                    replica_groups=[list(range(N_CORES))],
                    ins=[m_cc[:]], outs=[m_full[:]])
            else:
                for _c in range(N_CORES):
                    nc.sync.dma_start(out=m_full[_c * PS:(_c + 1) * PS, :], in_=m_cc[:])

            # ---- gather pipelines (two int16 windows)
            g_tiles = [[], []]
            gpools = [gp0, gp1]
            for w in range(2) if STAGE >= 3 else []:
                base = w * half
                for cidx in range(ncalls[w]):
                    gt = gpools[w].tile([P, GK, P], dt.bfloat16, name=f"g{w}")
                    # Stripe gather calls across the 4 SWDGE queues: the
                    # desc-gen ucode runs on Q7 core pair `queue_num`, so
                    # adjacent calls (in consumption order) land on
                    # different core pairs and generate concurrently.
                    nc.gpsimd.dma_gather(
                        out_ap=gt[:],
                        in_ap=m_full[base:base + half, :],
                        idxs_ap=idx_sb[w][:, cidx * NIDX // 16:(cidx + 1) * NIDX // 16],
                        num_idxs=NIDX, num_idxs_reg=NIDX, elem_size=P,
                        queue_num=2 * w + cidx % 2)
                    g_tiles[w].append(gt)

            # ---- aggregation + stats
            s1_parts = stp.tile([P, NT], dt.float32)
            s2_parts = stp.tile([P, NT], dt.float32)
            opre_all = stp.tile([P, NT, P], dt.float32)
            OHMAX = int(nbmax.sum(axis=1).max())
            for t in range(NT):
                if STAGE < 4:
                    op_t = opre_all[:, t, :]
                    nc.vector.memset(op_t, 0.125)
                    nc.vector.tensor_reduce(out=s1_parts[:, t:t + 1], in_=op_t,
                                            axis=mybir.AxisListType.X,
                                            op=mybir.AluOpType.add)
                    nc.vector.tensor_reduce(out=s2_parts[:, t:t + 1], in_=op_t,
                                            axis=mybir.AxisListType.X,
                                            op=mybir.AluOpType.add)
                    continue
                ps_t = aps.tile([P, P], dt.float32, name="agg")
                total_nb = int(nbmax[t, 0] + nbmax[t, 1])
                oh = ohp.tile([P, OHMAX, P], dt.bfloat16, name="oh")
                done = 0
                for w in range(2):
                    nbw = int(nbmax[t, w])
                    if nbw == 0:
                        continue
                    b0 = int(soff[t, w])
                    nc.vector.tensor_tensor(
                        out=oh[:, done:done + nbw, :],
                        in0=tgtl_sb[w][:, b0:b0 + nbw].unsqueeze(2)
                            .to_broadcast([P, nbw, P]),
                        in1=iota_sb[:].unsqueeze(1).to_broadcast([P, nbw, P]),
                        op=mybir.AluOpType.is_equal)
                    for b in range(nbw):
                        j = b0 + b
                        gt = g_tiles[w][j // GK]
                        nc.tensor.matmul(
                            out=ps_t[:], lhsT=gt[:, j % GK, :],
                            rhs=oh[:, done + b, :],
                            start=(done + b == 0),
                            stop=(done + b == total_nb - 1))
                    done += nbw
                op_t = opre_all[:, t, :]
                nc.vector.tensor_mul(out=op_t, in0=ps_t[:],
                                     in1=disb_sb[:, t * P:(t + 1) * P])
                nc.vector.tensor_reduce(out=s1_parts[:, t:t + 1], in_=op_t,
                                        axis=mybir.AxisListType.X,
                                        op=mybir.AluOpType.add)
                sq_t = ep.tile([P, P], dt.float32, name="sq")
                nc.scalar.activation(out=sq_t[:], in_=op_t,
                                     func=mybir.ActivationFunctionType.Square)
                nc.vector.tensor_reduce(out=s2_parts[:, t:t + 1], in_=sq_t[:],
                                        axis=mybir.AxisListType.X,
                                        op=mybir.AluOpType.add)

            # ---- BN stats allreduce + affine coefficients
            st_sb = stp.tile([P, 2], dt.float32)
            nc.vector.tensor_reduce(out=st_sb[:, 0:1], in_=s1_parts[:],
                                    axis=mybir.AxisListType.X, op=mybir.AluOpType.add)
            nc.vector.tensor_reduce(out=st_sb[:, 1:2], in_=s2_parts[:],
                                    axis=mybir.AxisListType.X, op=mybir.AluOpType.add)
            st_in = dram.tile([P, 2], dt.float32)
            st_out = dram.tile([P, 2], dt.float32, addr_space="Shared")
            st2_sb = stp.tile([P, 2], dt.float32)
            if STAGE >= 5:
                nc.sync.dma_start(out=st_in[:], in_=st_sb[:])
                nc.gpsimd.collective_compute(
                    "AllReduce", mybir.AluOpType.add,
                    replica_groups=[list(range(N_CORES))],
                    ins=[st_in[:]], outs=[st_out[:]])
                nc.sync.dma_start(out=st2_sb[:], in_=st_out[:])
            else:
                nc.vector.tensor_scalar_mul(st2_sb[:], st_sb[:], float(N_CORES))

            mean = stp.tile([P, 1], dt.float32)
            nc.vector.tensor_scalar_mul(mean[:], st2_sb[:, 0:1], INV_N)
            var = stp.tile([P, 1], dt.float32)
            nc.vector.tensor_scalar_mul(var[:], st2_sb[:, 1:2], INV_N)
            nmm = stp.tile([P, 1], dt.float32)
            nc.vector.scalar_tensor_tensor(out=nmm[:], in0=mean[:], scalar=-1.0,
                                           in1=mean[:], op0=mybir.AluOpType.mult,
                                           op1=mybir.AluOpType.mult)
            nc.vector.tensor_add(out=var[:], in0=var[:], in1=nmm[:])
            nc.vector.tensor_scalar_add(var[:], var[:], BN_EPS)
            std = stp.tile([P, 1], dt.float32)
            nc.scalar.activation(out=std[:], in_=var[:],
                                 func=mybir.ActivationFunctionType.Sqrt)
            rstd = stp.tile([P, 1], dt.float32)
            nc.vector.reciprocal(out=rstd[:], in_=std[:])
            A = stp.tile([P, 1], dt.float32)
            nc.vector.tensor_mul(out=A[:], in0=gamma_sb[:], in1=rstd[:])
            B = stp.tile([P, 1], dt.float32)
            nc.vector.tensor_mul(out=B[:], in0=A[:], in1=mean[:])
            nc.vector.scalar_tensor_tensor(out=B[:], in0=B[:], scalar=-1.0,
                                           in1=beta_sb[:], op0=mybir.AluOpType.mult,
                                           op1=mybir.AluOpType.add)

            # ---- finalize: relu(A*x + B), write feature-major output
            for t in range(NT):
                fin = ep.tile([P, P], dt.float32, name="fin")
                nc.scalar.activation(out=fin[:], in_=opre_all[:, t, :],
                                     func=mybir.ActivationFunctionType.Relu,
                                     bias=B[:], scale=A[:])
                nc.sync.dma_start(out=t_out[:, t * P:(t + 1) * P], in_=fin[:])

    nc.compile()
    return nc


# ---------------------------------------------------------------- entrypoint
def kernel(x, edge_index, W, b, gamma, beta):
    x = np.asarray(x, dtype=np.float32)
    edge_index = np.asarray(edge_index)
    W = np.asarray(W, dtype=np.float32)
    gamma = np.asarray(gamma, dtype=np.float32)
    beta = np.asarray(beta, dtype=np.float32)
    # bias cancels exactly under BatchNorm (constant per-feature shift); unused.

    plan, per_core = _plan_and_pack(x, edge_index, W, gamma, beta)
    nc = _build(plan)
    res = run_bass_kernel_spmd(nc, per_core, list(range(N_CORES)))

    N, shard = plan["N"], plan["shard"]
    out = np.empty((N, P), np.float32)
    for c in range(N_CORES):
        lo = c * shard
        hi = min((c + 1) * shard, N)
        out[lo:hi] = res.results[c]["out_t"][:, : hi - lo].T
    return out


if __name__ == "__main__":
    rng = np.random.default_rng(0)
    N, E = 2048, 8192
    x = rng.standard_normal((N, 256), dtype=np.float32)
    ei = rng.integers(0, N, (2, E)).astype(np.int64)
    W = (rng.standard_normal((256, 128), dtype=np.float32) / 16)
    g = rng.standard_normal(128).astype(np.float32) + 1.2
    be = rng.standard_normal(128).astype(np.float32)
    got = kernel(x=x, edge_index=ei, W=W, b=np.zeros(128, np.float32), gamma=g, beta=be)

    h = x @ W
    loops = np.arange(N)
    r2 = np.concatenate([ei[0], loops]); c2 = np.concatenate([ei[1], loops])
    deg = np.bincount(c2, minlength=N).astype(np.float32)
    dis = 1.0 / np.sqrt(deg)
    out = np.zeros((N, 128), np.float32)
    np.add.at(out, c2, h[r2] * (dis[r2] * dis[c2])[:, None])
    mean = out.mean(0); var = ((out - mean) ** 2).mean(0)
    ref = np.maximum(g * (out - mean) / np.sqrt(var + BN_EPS) + be, 0)
    err = np.abs(got - ref)
    print("absmax:", err.max(), "scale:", np.abs(ref).max(),
          "rel:", err.max() / np.abs(ref).max())

